# revision 31
# baseline (speedup 1.0000x reference)
"""Trainium2 Bass kernel for nn_MultiHeadAttention_85229331022244.

Computation (per batch b):
  xh = x.reshape(B,T,64,16); q/k/v = per-head 64x64 projections of xh
  q,k: interleaved RoPE over the FULL 1024-dim feature axis
  scores = q @ k.T / sqrt(1024)  (single attention map over full D)
  causal softmax; y = attn @ v

Sharding: core i -> batch i//2, q-block parity i%2 (even/odd 128-row q-blocks
interleaved between the two cores of a batch).  Every core runs an identical
program; parity differences are carried purely in DATA (a per-core key-block
permutation + 6 multiplicative mask tiles + per-core RoPE tables).

Dataflow (S-transposed flash):
  - heads reordered even-first and paired so projections are 8 block-diagonal
    128x128 matmuls producing K^T/Q^T in [feature, token] layout.
  - scores computed TRANSPOSED: S^T[key, q] tiles, so exp(S^T) is directly
    the lhsT of the attn@V matmuls -- no P transposes.
  - softmax row sums via N=1 matmuls against a ones vector.
  - causal masking via 6 multiplicative [128,128] masks (per-core data).
  - RoPE cos/sin tables fully precomputed on HOST and DMA'd (frees Vector).
  - Q projection inputs come from a host-packed contiguous xQ tensor so the
    Q matmuls run at N=256.
  - K/Q projections write a single 2-bank PSUM tile [128,2,512] drained by
    ONE Scalar activation; V copies run on GpSimd (Pool); y-lo on Scalar,
    y-hi on Vector.
  - emission interleaves next stripes' projection blocks between this pair's
    attn@V chains so PSUM-recycle drains hide behind PE work.
"""

import math
from contextlib import ExitStack

import numpy as np
import ml_dtypes

import concourse.bass as bass
import concourse.mybir as mybir
import concourse.tile as tile
from concourse import bacc
from concourse.bass import ts, ds

BF16 = ml_dtypes.bfloat16

D_MODEL = 1024
N_HEADS = 16
HEAD_D = 64
ROPE_BASE = 10000.0
GAMMA = 1.0 / math.sqrt(D_MODEL)
T = 4096
NSTR = T // 512  # 8 key stripes / q groups per core

HEAD_PAIRS = [(0, 2), (4, 6), (8, 10), (12, 14), (1, 3), (5, 7), (9, 11), (13, 15)]

V_COPY_ENGINE = "scalar"  # "gpsimd" (no PSUM access) | "scalar" | "vector"


def _feature_perm():
    perm = np.zeros(1024, dtype=np.int64)
    for c, (ha, hb) in enumerate(HEAD_PAIRS):
        for p in range(128):
            h = ha if p < 64 else hb
            perm[c * 128 + p] = (p % 64) * 16 + h
    return perm


PERM = _feature_perm()
INV_PERM = np.argsort(PERM)


def _block_weights(w):
    out = np.zeros((8, 128, 128), dtype=np.float32)
    for c, (ha, hb) in enumerate(HEAD_PAIRS):
        out[c, :64, :64] = w[:, :, ha]
        out[c, 64:, 64:] = w[:, :, hb]
    return out.astype(BF16)


def _freqs():
    p = np.arange(128)
    f = np.zeros((4, 128), dtype=np.float64)
    for c in range(4):
        fidx = (p % 64) * 8 + (2 * c + p // 64)
        f[c] = ROPE_BASE ** (-fidx / 512.0)
    return f


FREQS = _freqs()


def _kcols(parity):
    order = []
    for s in range(NSTR):
        if parity == 0:
            order += [4 * s + 1, 4 * s + 0, 4 * s + 3, 4 * s + 2]
        else:
            order += [4 * s + 0, 4 * s + 1, 4 * s + 2, 4 * s + 3]
    return np.concatenate([np.arange(128) + 128 * b for b in order])


def _msel(parity):
    r = np.arange(128)[:, None]
    c = np.arange(128)[None, :]
    tri = (r <= c).astype(np.float32)
    ones = np.ones((128, 128), np.float32)
    zeros = np.zeros((128, 128), np.float32)
    if parity == 0:
        m = [zeros, tri, zeros, zeros, zeros, tri]
    else:
        m = [ones, tri, zeros, zeros, ones, tri]
    return np.stack(m).astype(BF16)


# ------------------------- device program -------------------------


def build_nc():
    dt = mybir.dt
    nc = bacc.Bacc("TRN2", target_bir_lowering=False)
    xS = nc.dram_tensor("xS", [NSTR, 128, 8, 512], dt.bfloat16, kind="ExternalInput")
    xQ = nc.dram_tensor("xQ", [NSTR, 128, 8, 256], dt.bfloat16, kind="ExternalInput")
    csD = nc.dram_tensor("csD", [NSTR, 128, 4, 2, 512], dt.bfloat16,
                         kind="ExternalInput")
    qcsD = nc.dram_tensor("qcsD", [NSTR, 128, 4, 2, 256], dt.bfloat16,
                          kind="ExternalInput")
    # packed bf16 constants: wq(1024) | wk(1024) | wv(1024) | msel(768)
    constsD = nc.dram_tensor("consts", [128, 3840], dt.bfloat16,
                             kind="ExternalInput")
    y = nc.dram_tensor("y", [2 * NSTR, 128, 1024], dt.bfloat16,
                       kind="ExternalOutput")

    with tile.TileContext(nc) as tc, ExitStack() as ctx:
        const = ctx.enter_context(tc.tile_pool(name="const", bufs=1))
        kv = ctx.enter_context(tc.tile_pool(name="kv", bufs=1))
        xpool = ctx.enter_context(tc.tile_pool(name="xpool", bufs=2))
        xqpool = ctx.enter_context(tc.tile_pool(name="xqpool", bufs=2))
        cspool = ctx.enter_context(tc.tile_pool(name="cspool", bufs=2))
        qcspool = ctx.enter_context(tc.tile_pool(name="qcspool", bufs=2))
        qpool = ctx.enter_context(tc.tile_pool(name="qpool", bufs=2))
        rtmp = ctx.enter_context(tc.tile_pool(name="rtmp", bufs=2))
        ptpool = ctx.enter_context(tc.tile_pool(name="ptpool", bufs=1))
        ypool = ctx.enter_context(tc.tile_pool(name="ypool", bufs=2))
        lpool = ctx.enter_context(tc.tile_pool(name="lpool", bufs=2))
        psS = ctx.enter_context(tc.tile_pool(name="psS", bufs=2, space="PSUM"))
        psY = ctx.enter_context(tc.tile_pool(name="psY", bufs=1, space="PSUM"))
        psL = ctx.enter_context(tc.tile_pool(name="psL", bufs=1, space="PSUM"))
        psP = ctx.enter_context(tc.tile_pool(name="psP", bufs=1, space="PSUM"))
        psV = ctx.enter_context(tc.tile_pool(name="psV", bufs=1, space="PSUM"))

        # ---- constants ----
        ones = const.tile([128, 1], dt.bfloat16, tag="ones", name="ones")
        nc.gpsimd.memset(ones[:], 1.0)
        # warm the ACT function tables (Copy/Exp) so the ~2.7us table load
        # overlaps the input DMAs instead of stalling the first projection.
        warm = const.tile([128, 1], dt.float32, tag="warm", name="warm")
        nc.scalar.activation(warm[:], ones[:],
                             mybir.ActivationFunctionType.Exp)
        # consts split into 4 DMAs so the first blocks can start as soon as
        # their weights land (wq first, msel last).
        cbq = const.tile([128, 1024], dt.bfloat16, tag="cbq", name="cbq")
        cbk = const.tile([128, 1024], dt.bfloat16, tag="cbk", name="cbk")
        cbv = const.tile([128, 1024], dt.bfloat16, tag="cbv", name="cbv")
        cbm = const.tile([128, 768], dt.bfloat16, tag="cbm", name="cbm")
        nc.sync.dma_start(cbq[:], constsD[:, 0:1024])
        wq_sb = [cbq[:, ds(128 * c, 128)] for c in range(8)]
        wk_sb = [cbk[:, ds(128 * c, 128)] for c in range(8)]
        wv_sb = [cbv[:, ds(128 * c, 128)] for c in range(8)]
        msel = [cbm[:, ds(128 * i, 128)] for i in range(6)]

        # resident K^T in fp8 chunk-pair layout [128, 2, 512] per (cp, s):
        # [:, 0, :] = chunk cp, [:, 1, :] = chunk cp+4, values scaled by 32.
        KT8 = {}
        for s in range(NSTR):
            for cp in range(4):
                KT8[(cp, s)] = kv.tile([128, 2, 512], dt.float8e4,
                                       tag=f"kt8{cp}_{s}", name=f"kt8{cp}_{s}")
        V = [kv.tile([128, 1024], dt.bfloat16, tag=f"v{kb}", name=f"v{kb}")
             for kb in range(4 * NSTR)]

        def rope6(out_e, out_o, ke, ko, cos, sin, w):
            """out_e = ke*cos - ko*sin ; out_o = ke*sin + ko*cos (width w)."""
            ta = rtmp.tile([128, 512], dt.bfloat16, tag="ta", name="ta")
            tb = rtmp.tile([128, 512], dt.bfloat16, tag="tb", name="tb")
            nc.vector.tensor_mul(ta[:, :w], ke[:, :w], cos)
            nc.vector.tensor_mul(tb[:, :w], ko[:, :w], sin)
            nc.vector.tensor_sub(out_e, ta[:, :w], tb[:, :w])
            ta2 = rtmp.tile([128, 512], dt.bfloat16, tag="ta", name="ta")
            tb2 = rtmp.tile([128, 512], dt.bfloat16, tag="tb", name="tb")
            nc.vector.tensor_mul(ta2[:, :w], ke[:, :w], sin)
            nc.vector.tensor_mul(tb2[:, :w], ko[:, :w], cos)
            nc.vector.tensor_add(out_o, ta2[:, :w], tb2[:, :w])

        QT8 = {}
        FP8_SCALE = 32.0

        def stripe_blocks(s, mid_dma=None, prologue=False):
            """Issue stripe s input DMAs now; return projection blocks.

            Each block emits one PSUM-tile's worth of matmuls plus its drain
            (and RoPE).  Block order alternates psP (K/Q) and psV (V) banks
            so PE never waits on a drain of the bank it just used.
            """
            xq = xqpool.tile([128, 8, 256], dt.bfloat16, tag="xq", name="xq")
            nc.sync.dma_start(xq[:], xQ[s])
            qcs = qcspool.tile([128, 4, 2, 256], dt.bfloat16, tag="qcs",
                               name="qcs")
            nc.sync.dma_start(qcs[:], qcsD[s])
            if mid_dma is not None:
                mid_dma()
            xt = xpool.tile([128, 8, 512], dt.bfloat16, tag="xt", name="xt")
            nc.sync.dma_start(xt[:], xS[s])
            cs = cspool.tile([128, 4, 2, 512], dt.bfloat16, tag="cs", name="cs")
            nc.sync.dma_start(cs[:], csD[s])

            def kblock(cp):
                def go():
                    pk = psP.tile([128, 2, 512], dt.float32, tag="pk",
                                  name="pk")
                    nc.tensor.matmul(pk[:, 0, :], lhsT=wk_sb[cp],
                                     rhs=xt[:, cp, :], start=True, stop=True)
                    nc.tensor.matmul(pk[:, 1, :], lhsT=wk_sb[cp + 4],
                                     rhs=xt[:, cp + 4, :], start=True,
                                     stop=True)
                    keko = rtmp.tile([128, 2, 512], dt.bfloat16, tag="keko",
                                     name="keko")
                    nc.scalar.activation(keko[:], pk[:],
                                         mybir.ActivationFunctionType.Copy,
                                         scale=FP8_SCALE)
                    rope6(KT8[(cp, s)][:, 0, :], KT8[(cp, s)][:, 1, :],
                          keko[:, 0, :], keko[:, 1, :],
                          cs[:, cp, 0, :], cs[:, cp, 1, :], 512)
                return go

            def qblock(cp, alt=False):
                def go():
                    # alt=True borrows the (momentarily idle) psV bank so
                    # back-to-back q-blocks overlap their Scalar drains
                    # (prologue q-phase only, before xt lands).
                    keko = rtmp.tile([128, 2, 512], dt.bfloat16, tag="keko",
                                     name="keko")
                    if alt:
                        pq = psV.tile([128, 512], dt.float32, tag="pv",
                                      name="pv")
                        pe_, po_ = pq[:, 0:256], pq[:, 256:512]
                        ke_, ko_ = keko[:, 0, 0:256], keko[:, 0, 256:512]
                        nc.tensor.matmul(pe_, lhsT=wq_sb[cp],
                                         rhs=xq[:, cp, :], start=True,
                                         stop=True)
                        nc.tensor.matmul(po_, lhsT=wq_sb[cp + 4],
                                         rhs=xq[:, cp + 4, :], start=True,
                                         stop=True)
                        nc.scalar.activation(keko[:, 0, :], pq[:],
                                             mybir.ActivationFunctionType.Copy,
                                             scale=FP8_SCALE)
                    else:
                        pk = psP.tile([128, 2, 512], dt.float32, tag="pk",
                                      name="pk")
                        pe_, po_ = pk[:, 0, :256], pk[:, 1, :256]
                        ke_, ko_ = keko[:, 0, :256], keko[:, 1, :256]
                        nc.tensor.matmul(pe_, lhsT=wq_sb[cp],
                                         rhs=xq[:, cp, :], start=True,
                                         stop=True)
                        nc.tensor.matmul(po_, lhsT=wq_sb[cp + 4],
                                         rhs=xq[:, cp + 4, :], start=True,
                                         stop=True)
                        nc.scalar.activation(keko[:, :, :256], pk[:, :, :256],
                                             mybir.ActivationFunctionType.Copy,
                                             scale=FP8_SCALE)
                    if s % 2 == 0:
                        QT8[cp] = qpool.tile([128, 2, 512], dt.float8e4,
                                             tag=f"qt8{cp}", name=f"qt8{cp}")
                    half = ds(256 * (s % 2), 256)
                    rope6(QT8[cp][:, 0, half], QT8[cp][:, 1, half],
                          ke_, ko_,
                          qcs[:, cp, 0, :], qcs[:, cp, 1, :], 256)
                return go

            def vblock(j, half):
                def go():
                    kb = 4 * s + j
                    pv = psV.tile([128, 512], dt.float32, tag="pv", name="pv")
                    for cc in range(4):
                        c = 4 * half + cc
                        nc.tensor.matmul(
                            pv[:, ts(cc, 128)],
                            lhsT=xt[:, c, ds(128 * j, 128)], rhs=wv_sb[c],
                            start=True, stop=True)
                    dst = V[kb][:, ds(512 * half, 512)]
                    # early stripes run in the Scalar-congested startup
                    # region: route a couple of copies to Vector instead
                    if s < 4 and half == 1 and j < 2:
                        nc.vector.tensor_copy(dst, pv[:])
                    else:
                        nc.scalar.activation(
                            dst, pv[:], mybir.ActivationFunctionType.Copy)
                return go

            # alternate psP blocks (K, Q) with psV blocks; Q first so the
            # next pair's QT8 (Vector rope) is ready before its scores.
            # In the prologue the q-blocks run back-to-back before xt lands,
            # so alternate them between psP and the then-idle psV bank.
            blocks = []
            for cp in range(4):
                blocks.append(qblock(cp, alt=(prologue and cp % 2 == 1)))
                blocks.append(vblock(cp, 0))
                blocks.append(kblock(cp))
                blocks.append(vblock(cp, 1))
            return blocks

        def emit_scores(u, blocks=()):
            """S^T tiles + exp + causal masking for pair u. Returns pts.

            `blocks` are next-stripe projection blocks interleaved between
            s_tiles (they rebind QT8, so we snapshot pair u's tiles first).
            """
            g0, g1 = 2 * u, 2 * u + 1
            pts = {}
            qt = dict(QT8)  # pair u's tiles; blocks rebind QT8 for pair u+1
            exp_scale = GAMMA / (FP8_SCALE * FP8_SCALE)
            blocks = list(blocks)

            def s_tile(kb, qofs, w):
                S = psS.tile([128, 512], dt.float32, tag="S", name="S")
                for cp in range(4):
                    nc.tensor.matmul(
                        S[:, :w],
                        lhsT=KT8[(cp, kb // 4)][:, :, ts(kb % 4, 128)],
                        rhs=qt[cp][:, :, ds(qofs, w)],
                        start=(cp == 0), stop=(cp == 3),
                        perf_mode=mybir.MatmulPerfMode.DoubleRow)
                pt = ptpool.tile([128, 512], dt.bfloat16, tag=f"pt{kb}",
                                 name=f"pt{kb}")
                nc.scalar.activation(pt[:, ds(qofs, w)], S[:, :w],
                                     mybir.ActivationFunctionType.Exp,
                                     scale=exp_scale)
                pts[kb] = pt

            n_tiles = 8 * u + 8
            nper = (len(blocks) + n_tiles - 1) // n_tiles if blocks else 0
            bi = 0

            def drip():
                nonlocal bi
                for b in blocks[bi:bi + nper]:
                    b()
                bi += nper

            for kb in range(8 * u + 4):
                s_tile(kb, 0, 512)
                drip()
            for j in range(4):
                kb = 8 * u + 4 + j
                if j < 2:
                    s_tile(kb, 256, 256)
                else:
                    s_tile(kb, 384, 128)  # g1 m0 half fully masked
                drip()
            for b in blocks[bi:]:
                b()
            # causal masking multiplies (GpSimd: all-SBUF, engine is idle)
            for g, base in ((g0, 0), (g1, 256)):
                for j in range(2):
                    kb = 4 * g + j
                    sl = ds(base, 128)
                    nc.gpsimd.tensor_mul(pts[kb][:, sl], pts[kb][:, sl],
                                         msel[j])
                for jj, j in enumerate((2, 3)):
                    kb = 4 * g + j
                    sl = ds(base + 128, 128)
                    nc.gpsimd.tensor_mul(pts[kb][:, sl], pts[kb][:, sl],
                                         msel[4 + jj])
            return pts

        def attnv_chain(pts, g, base, m, L, alt=False):
            """One attn@V accumulation chain + normalize + store.

            alt=True borrows the (idle) psP banks for Ylo/Yhi so consecutive
            chains' PSUM drains overlap -- only valid when no projection
            blocks are interleaved nearby (the last pair).
            """
            if alt:
                pkt = psP.tile([128, 2, 512], dt.float32, tag="pk", name="pk")
                Ylo, Yhi = pkt[:, 0, :], pkt[:, 1, :]
            else:
                Ylo = psY.tile([128, 512], dt.float32, tag="Ylo", name="Ylo")
                Yhi = psY.tile([128, 512], dt.float32, tag="Yhi", name="Yhi")
            last = 4 * g + 1 if m == 0 else 4 * g + 3
            for kb in range(last + 1):
                lhs = pts[kb][:, ds(base + 128 * m, 128)]
                nc.tensor.matmul(Ylo[:], lhsT=lhs, rhs=V[kb][:, 0:512],
                                 start=(kb == 0), stop=(kb == last))
                nc.tensor.matmul(Yhi[:], lhsT=lhs, rhs=V[kb][:, 512:1024],
                                 start=(kb == 0), stop=(kb == last))
                nc.tensor.matmul(L[:, ds(m, 1)], lhsT=lhs, rhs=ones[:],
                                 start=(kb == 0), stop=(kb == last))
            linv = lpool.tile([128, 1], dt.float32, tag="li", name="li")
            nc.vector.reciprocal(linv[:], L[:, ds(m, 1)])
            y_sb = ypool.tile([128, 1024], dt.bfloat16, tag="y", name="y")
            nc.scalar.activation(y_sb[:, 0:512], Ylo[:],
                                 mybir.ActivationFunctionType.Copy,
                                 scale=linv[:])
            nc.vector.tensor_scalar_mul(y_sb[:, 512:1024], Yhi[:], linv[:])
            nc.sync.dma_start(y[2 * g + m], y_sb[:])

        # -------- main pipeline --------
        # prologue: weight DMAs land first; q-blocks run while xt/cs DMAs
        # stream, then (psP, psV) pairs alternate between the two stripes
        b0 = stripe_blocks(0, mid_dma=lambda: (
            nc.sync.dma_start(cbk[:], constsD[:, 1024:2048]),
            nc.sync.dma_start(cbv[:], constsD[:, 2048:3072])),
            prologue=True)
        b1 = stripe_blocks(1, prologue=True)
        nc.sync.dma_start(cbm[:], constsD[:, 3072:3840])
        # b layout: [q0,v00,k0,v01, q1,v10,k1,v11, ...]
        for i in range(0, 16, 4):
            b0[i]()          # q-blocks of stripe 0 first (xq/qcs land early)
        for i in range(0, 16, 4):
            b1[i]()
        for i in range(0, 16, 4):
            b0[i + 1]()
            b0[i + 2]()
            b0[i + 3]()
            b1[i + 1]()
            b1[i + 2]()
            b1[i + 3]()

        for u in range(NSTR // 2):
            g0, g1 = 2 * u, 2 * u + 1
            if u + 1 < NSTR // 2:
                nxt = stripe_blocks(2 * u + 2) + stripe_blocks(2 * u + 3)
            else:
                nxt = []
            # keep 3 blocks per inter-chain gap (psY-recycle cover), rest
            # dripped through the scores phase
            n_gap = min(9, len(nxt))
            pts = emit_scores(u, nxt[:len(nxt) - n_gap])
            gap_blocks = nxt[len(nxt) - n_gap:]
            chains = []
            for g, base in ((g0, 0), (g1, 256)):
                L = psL.tile([128, 2], dt.float32, tag="L", name="L")
                for m in range(2):
                    chains.append((g, base, m, L))
            per = 3
            for i, (g, base, m, L) in enumerate(chains):
                attnv_chain(pts, g, base, m, L, alt=(not nxt and i % 2 == 1))
                for b in gap_blocks[i * per:(i + 1) * per]:
                    b()

    nc.compile()
    return nc


# ------------------------- host side -------------------------


def _cs_tables(parity):
    """Host-precomputed RoPE tables in this core's key order.

    Returns (csS, qcsS):
      csS  [NSTR, 128, 4, 2, 512] bf16 -- cos/sin(f_cp[p] * t) at the K
           columns of each stripe (core key order).
      qcsS [NSTR, 128, 4, 2, 256] bf16 -- same at the q columns (slots 1,3).
    """
    kc = _kcols(parity).astype(np.float64)  # global t per core column
    csS = np.zeros((NSTR, 128, 4, 2, 512), np.float64)
    for s in range(NSTR):
        t = kc[512 * s:512 * (s + 1)]
        for cp in range(4):
            ang = FREQS[cp][:, None] * t[None, :]
            csS[s, :, cp, 0, :] = np.cos(ang)
            csS[s, :, cp, 1, :] = np.sin(ang)
    qcsS = np.concatenate([csS[:, :, :, :, 128:256], csS[:, :, :, :, 384:512]],
                          axis=4)
    return csS.astype(BF16), np.ascontiguousarray(qcsS).astype(BF16)


def prep_core_inputs(xb, w2q, w2k, w2v, parity):
    """Inputs for one core: batch slice xb (T, 1024) fp32, parity 0/1."""
    kc = _kcols(parity)
    xpT = np.ascontiguousarray(xb.T[PERM]).reshape(8, 128, T)
    xperm = xpT[:, :, kc]
    xS = np.ascontiguousarray(
        xperm.reshape(8, 128, NSTR, 512).transpose(2, 1, 0, 3)).astype(BF16)
    # q columns (slots 1,3 of each stripe) packed contiguously
    xQ = np.ascontiguousarray(
        np.concatenate([xS[:, :, :, 128:256], xS[:, :, :, 384:512]], axis=3))
    csS, qcsS = _CS_CACHE[parity]
    consts = np.concatenate([
        w2q.transpose(1, 0, 2).reshape(128, 1024),
        w2k.transpose(1, 0, 2).reshape(128, 1024),
        w2v.transpose(1, 0, 2).reshape(128, 1024),
        _msel(parity).transpose(1, 0, 2).reshape(128, 768),
    ], axis=1).astype(BF16)
    return {
        "xS": xS,
        "xQ": xQ,
        "csD": csS,
        "qcsD": qcsS,
        "consts": np.ascontiguousarray(consts),
    }


_CS_CACHE = {0: None, 1: None}
_NC_CACHE = {}
last_in_maps = None
last_nc = None


def kernel(x, w_q, w_k, w_v):
    global last_in_maps, last_nc
    from concourse.bass_utils import run_bass_kernel_spmd

    B, Tx, D = x.shape
    assert (B, Tx, D) == (4, 4096, 1024)
    x = np.asarray(x, dtype=np.float32)
    w2q = _block_weights(np.asarray(w_q, dtype=np.float32))
    w2k = _block_weights(np.asarray(w_k, dtype=np.float32))
    w2v = _block_weights(np.asarray(w_v, dtype=np.float32))

    for parity in range(2):
        if _CS_CACHE[parity] is None:
            _CS_CACHE[parity] = _cs_tables(parity)

    in_maps = []
    for core in range(8):
        b, parity = core // 2, core % 2
        in_maps.append(prep_core_inputs(x[b], w2q, w2k, w2v, parity))
    last_in_maps = in_maps

    if "nc" not in _NC_CACHE:
        _NC_CACHE["nc"] = build_nc()
    nc = _NC_CACHE["nc"]
    last_nc = nc

    res = run_bass_kernel_spmd(nc, in_maps, core_ids=list(range(8)))
    out = np.zeros((B, Tx, D), dtype=np.float32)
    for core in range(8):
        b, parity = core // 2, core % 2
        yk = res.results[core]["y"].astype(np.float32)  # [16, 128, 1024]
        for g in range(NSTR):
            for m in range(2):
                G = 4 * g + 2 * m + parity
                out[b, 128 * G:128 * (G + 1), :] = yk[2 * g + m][:, INV_PERM]
    return out
